# revision 7
# baseline (speedup 1.0000x reference)
"""Multi-head attention (bs=2, len=2048, d_model=1024, 16 heads) on 8 TRN2 cores.

Sharding: batch across 2 groups of 4 cores; 4 heads per core within a group.
Per core:
  - inputs fed host-transposed: qT/kT/vT [1024, 2048] (d_model on partitions)
  - projections qhT/khT/vhT [256, 2048] computed with psum-resident accumulation
  - vhT PE-transposed into V_ext [2048, 4*65] (65th col = ones -> colsum ride-along)
  - per (q-block 512, head): S^T = K_h Q_h^T in [k, q] layout, exp on ACT -> P^T,
    O_ext = [V|1]^T P^T accumulated over k (row 64 = softmax denominators)
  - P^T blocks PE-transposed back to [q, k]; PSUM->SBUF copy fused with the
    1/denominator per-partition scale -> attn output written in natural layout
  - O normalized via PE broadcast of the reciprocal row, fc partials per q-block
    ReduceScattered (add) within the 4-core group -> each core owns a 128-row strip
  - residual added post-RS; host reassembles strips.
All matmuls in float32r (TF32-like fast path) except the fp32 broadcast;
PE transposes are bit-exact permutations.
"""

import sys

sys.path.insert(0, "/opt/trn_rl_repo")

import numpy as np

import concourse.bacc as bacc
import concourse.tile as tile
from concourse import mybir
from concourse.bass_utils import run_bass_kernel_spmd

F32 = mybir.dt.float32
F32R = mybir.dt.float32r
AF = mybir.ActivationFunctionType

BS = 2
DM = 1024
NH = 16
DK = 64
NCORES = 8
GROUPS = [[0, 1, 2, 3], [4, 5, 6, 7]]
HPC = 4          # heads per core
RPG = 4          # ranks per group


def build_nc(LEN=2048):
    NKI = LEN // 128       # k chunks
    NQB = LEN // 512       # q blocks
    NDC = DM // 128        # d_model chunks
    WCOLS = HPC * DK       # 256: projection output columns for this core's heads

    nc = bacc.Bacc("TRN2", target_bir_lowering=False)

    qT = nc.declare_dram_parameter("qT", [DM, LEN], F32R, isOutput=False)
    kT = nc.declare_dram_parameter("kT", [DM, LEN], F32R, isOutput=False)
    vT = nc.declare_dram_parameter("vT", [DM, LEN], F32R, isOutput=False)
    wq = nc.declare_dram_parameter("wq", [DM, WCOLS], F32R, isOutput=False)
    wk = nc.declare_dram_parameter("wk", [DM, WCOLS], F32R, isOutput=False)
    wv = nc.declare_dram_parameter("wv", [DM, WCOLS], F32R, isOutput=False)
    wo = nc.declare_dram_parameter("wo", [WCOLS, DM], F32R, isOutput=False)
    qres = nc.declare_dram_parameter("qres", [NQB * 128, DM], F32, isOutput=False)
    ident = nc.declare_dram_parameter("ident", [128, 128], F32, isOutput=False)
    onesv = nc.declare_dram_parameter("onesv", [128, HPC], F32R, isOutput=False)

    attn_o = nc.declare_dram_parameter("attn_o", [HPC, LEN, LEN], F32, isOutput=True)
    out_o = nc.declare_dram_parameter("out_o", [NQB * 128, DM], F32, isOutput=True)

    with tile.TileContext(nc) as tc:
        with (
            tc.tile_pool(name="sb", bufs=1) as sb,
            tc.tile_pool(name="dram", bufs=1, space="DRAM") as dram,
        ):
            identF = sb.tile([128, 128], F32, tag="identF")
            nc.sync.dma_start(identF[:], ident[:])
            identR = sb.tile([128, 128], F32R, tag="identR")
            nc.sync.dma_start(identR[:], ident[:].bitcast(F32R))
            onesF = sb.tile([65, 128], F32, tag="onesF")
            nc.vector.memset(onesF[:], 1.0)

            # per-head Wo rows at partition base 0 for the fc
            wo4 = []
            for h in range(HPC):
                t = sb.tile([64, DM], F32R, tag="wo4", bufs=HPC, name=f"wo4_{h}")
                nc.sync.dma_start(t[:], wo[h * 64:(h + 1) * 64, :])
                wo4.append(t)

            # ---------------- Phase 1: projections ----------------
            qhT = [None, None]   # [128, LEN] heads (0,1) and (2,3), dims on partitions
            khT = [None, None]
            vhT = [None, None]   # F32, pre-transpose staging

            with tc.tile_pool(name="psp", bufs=2, space="PSUM") as psp:
                for name, src, w in (("q", qT, wq), ("k", kT, wk), ("v", vT, wv)):
                    pp = [psp.tile([128, LEN], F32, tag="proj", name=f"pp_{name}{t}")
                          for t in range(2)]
                    for d in range(NDC):
                        wt = sb.tile([128, WCOLS], F32R, tag="w", bufs=3,
                                     name=f"w_{name}{d}")
                        nc.sync.dma_start(wt[:], w[d * 128:(d + 1) * 128, :])
                        xt = sb.tile([128, LEN], F32R, tag="xs", bufs=3,
                                     name=f"x_{name}{d}")
                        nc.sync.dma_start(xt[:], src[d * 128:(d + 1) * 128, :])
                        for t in range(2):
                            for qc in range(LEN // 512):
                                nc.tensor.matmul(
                                    pp[t][:, qc * 512:(qc + 1) * 512],
                                    wt[:, t * 128:(t + 1) * 128],
                                    xt[:, qc * 512:(qc + 1) * 512],
                                    start=(d == 0), stop=(d == NDC - 1),
                                )
                    for t in range(2):
                        if name == "v":
                            # reuse the streaming-tile slots (xs is idle after
                            # the v d-loop; these are its last allocations)
                            vhT[t] = sb.tile([128, LEN], F32, tag="xs", bufs=3,
                                             name=f"vhT{t}")
                            nc.scalar.copy(vhT[t][:], pp[t][:])
                        else:
                            dst = sb.tile([128, LEN], F32R, tag=f"{name}hT{t}",
                                          name=f"{name}hT{t}")
                            nc.scalar.copy(dst[:], pp[t][:])
                            (qhT if name == "q" else khT)[t] = dst

            # ---------------- attn-phase PSUM pools ----------------
            with (
                tc.tile_pool(name="pss", bufs=2, space="PSUM") as pss,    # S / fc
                tc.tile_pool(name="pso", bufs=2, space="PSUM") as pso,    # O_ext
                tc.tile_pool(name="psb1", bufs=1, space="PSUM") as psb1,  # bcast
                tc.tile_pool(name="psrc", bufs=1, space="PSUM") as psrc,  # rcpT
                tc.tile_pool(name="pst", bufs=2, space="PSUM") as pst,    # transposes
            ):
                # V_ext: [128 seq, 4*65] per seq chunk; col h*65+64 = ones
                vext = []
                for sc in range(NKI):
                    t = sb.tile([128, HPC * 65], F32R, tag="vext", bufs=NKI,
                                name=f"vext{sc}")
                    nc.sync.dma_start(t[:, 64::65], onesv[:])
                    vext.append(t)
                for t in range(2):
                    for scg in range(NKI // 4):
                        tp = pst.tile([128, 512], F32, tag="t", name=f"vt{t}_{scg}")
                        for j in range(4):
                            sc = scg * 4 + j
                            nc.tensor.matmul(
                                tp[:, j * 128:(j + 1) * 128],
                                vhT[t][:, sc * 128:(sc + 1) * 128],
                                identF[:], is_transpose=True,
                                start=(j == 0), stop=(j == 3),
                            )
                        for j in range(4):
                            sc = scg * 4 + j
                            for hh in range(2):
                                h = 2 * t + hh
                                nc.scalar.copy(
                                    vext[sc][:, h * 65:h * 65 + 64],
                                    tp[:, j * 128 + hh * 64:j * 128 + hh * 64 + 64],
                                )

                # ---------------- Phase 2: attention ----------------
                for qb in range(NQB):
                    otn = []   # per head normalized O^T [64, 512]
                    for h in range(HPC):
                        hp, hoff = h // 2, (h % 2) * 64
                        o_ps = pso.tile([65, 512], F32, tag="o", name=f"o{qb}_{h}")
                        pts = []
                        for ki in range(NKI):
                            s_ps = pss.tile([128, 512], F32, tag="s",
                                            name=f"s{qb}_{h}_{ki}")
                            nc.tensor.matmul(
                                s_ps[:],
                                khT[hp][hoff:hoff + 64, ki * 128:(ki + 1) * 128],
                                qhT[hp][hoff:hoff + 64, qb * 512:(qb + 1) * 512],
                                start=True, stop=True,
                            )
                            pt = sb.tile([128, 512], F32R, tag="pt", bufs=18,
                                         name=f"pt{qb}_{h}_{ki}")
                            nc.scalar.activation(pt[:], s_ps[:], AF.Exp, scale=0.125)
                            nc.tensor.matmul(
                                o_ps[:],
                                vext[ki][:, h * 65:(h + 1) * 65],
                                pt[:],
                                start=(ki == 0), stop=(ki == NKI - 1),
                            )
                            pts.append(pt)

                        # reciprocals of colsums (row 64), still q-on-free
                        rrow = sb.tile([65, 512], F32, tag="rrow", bufs=2,
                                       name=f"rrow{qb}_{h}")
                        nc.vector.reciprocal(rrow[64:65, :], o_ps[64:65, :])
                        # broadcast recip across partitions 0..63 (fp32, exact)
                        bc = psb1.tile([64, 512], F32, tag="bc", name=f"bc{qb}_{h}")
                        nc.tensor.matmul(bc[:], onesF[64:65, 0:64],
                                         rrow[64:65, :], start=True, stop=True)
                        bcs = sb.tile([64, 512], F32, tag="bcs", bufs=2,
                                      name=f"bcs{qb}_{h}")
                        nc.scalar.copy(bcs[:], bc[:])
                        on = sb.tile([64, 512], F32R, tag="otn", bufs=HPC + 1,
                                     name=f"otn{qb}_{h}")
                        nc.vector.tensor_mul(on[:], o_ps[0:64, :], bcs[:])
                        otn.append(on)
                        # transpose recip row -> [128, 4] q-on-partition
                        rc_ps = psrc.tile([128, 4], F32, tag="rc", name=f"rc{qb}_{h}")
                        for qs in range(4):
                            nc.tensor.matmul(
                                rc_ps[:, qs:qs + 1],
                                rrow[64:65, qs * 128:(qs + 1) * 128],
                                onesF[64:65, 0:1], is_transpose=True,
                                start=(qs == 0), stop=(qs == 3),
                            )
                        rcp = sb.tile([128, 4], F32, tag="rcp", bufs=2,
                                      name=f"rcp{qb}_{h}")
                        nc.scalar.copy(rcp[:], rc_ps[:])

                        # transpose P^T -> [q, k] blocks, scale-copy, DMA out
                        for qs in range(4):
                            asb = sb.tile([128, LEN], F32, tag="attn", bufs=3,
                                          name=f"attn{qb}_{h}_{qs}")
                            for kig in range(NKI // 4):
                                tp = pst.tile([128, 512], F32R, tag="t",
                                              name=f"t{qb}_{h}_{qs}_{kig}")
                                for j in range(4):
                                    ki = kig * 4 + j
                                    nc.tensor.matmul(
                                        tp[:, j * 128:(j + 1) * 128],
                                        pts[ki][:, qs * 128:(qs + 1) * 128],
                                        identR[:], is_transpose=True,
                                        start=(j == 0), stop=(j == 3),
                                    )
                                nc.vector.tensor_scalar_mul(
                                    asb[:, kig * 512:(kig + 1) * 512],
                                    tp[:].bitcast(F32),
                                    rcp[:, qs:qs + 1],
                                )
                            nc.sync.dma_start(
                                attn_o[h, qb * 512 + qs * 128:qb * 512 + (qs + 1) * 128, :],
                                asb[:],
                            )

                    # ---- fc partial for this q-block + ReduceScatter ----
                    partial = dram.tile([512, DM], F32, tag="partial", bufs=NQB,
                                        name=f"partial{qb}")
                    for qs in range(4):
                        psb = sb.tile([128, DM], F32, tag="psb", bufs=2,
                                      name=f"psb{qb}_{qs}")
                        for dm_i in range(2):
                            fp = pss.tile([128, 512], F32, tag="s",
                                          name=f"fc{qb}_{qs}_{dm_i}")
                            for h in range(HPC):
                                nc.tensor.matmul(
                                    fp[:],
                                    otn[h][:, qs * 128:(qs + 1) * 128],
                                    wo4[h][:, dm_i * 512:(dm_i + 1) * 512],
                                    start=(h == 0), stop=(h == HPC - 1),
                                )
                            nc.scalar.copy(psb[:, dm_i * 512:(dm_i + 1) * 512], fp[:])
                        nc.sync.dma_start(
                            partial[qs * 128:(qs + 1) * 128, :], psb[:])
                    rsout = dram.tile([128, DM], F32, tag="rsout", bufs=NQB,
                                      name=f"rsout{qb}")
                    nc.gpsimd.collective_compute(
                        "ReduceScatter", mybir.AluOpType.add,
                        replica_groups=GROUPS,
                        ins=[partial.opt()], outs=[rsout.opt()],
                    )
                    rsb = sb.tile([128, DM], F32, tag="rsb", bufs=2,
                                  name=f"rsb{qb}")
                    nc.sync.dma_start(rsb[:], rsout[:])
                    qrt = sb.tile([128, DM], F32, tag="qrt", bufs=2,
                                  name=f"qrt{qb}")
                    nc.sync.dma_start(qrt[:], qres[qb * 128:(qb + 1) * 128, :])
                    osb = sb.tile([128, DM], F32, tag="osb", bufs=2,
                                  name=f"osb{qb}")
                    nc.vector.tensor_add(osb[:], rsb[:], qrt[:])
                    nc.sync.dma_start(out_o[qb * 128:(qb + 1) * 128, :], osb[:])

    nc.compile()
    return nc


_NC_CACHE = {}


def _get_nc(LEN):
    if LEN not in _NC_CACHE:
        _NC_CACHE[LEN] = build_nc(LEN)
    return _NC_CACHE[LEN]


def make_in_maps(q, k, v, Wq, Wk, Wv, Wo, LEN):
    NQB = LEN // 512
    ident = np.eye(128, dtype=np.float32)
    in_maps = []
    for c in range(NCORES):
        b, rank = c // RPG, c % RPG
        h0 = rank * HPC
        cols = slice(h0 * DK, (h0 + HPC) * DK)
        strips = np.concatenate(
            [q[b, qb * 512 + rank * 128: qb * 512 + (rank + 1) * 128, :]
             for qb in range(NQB)], axis=0)
        in_maps.append(dict(
            qT=np.ascontiguousarray(q[b].T),
            kT=np.ascontiguousarray(k[b].T),
            vT=np.ascontiguousarray(v[b].T),
            wq=np.ascontiguousarray(Wq[:, cols]),
            wk=np.ascontiguousarray(Wk[:, cols]),
            wv=np.ascontiguousarray(Wv[:, cols]),
            wo=np.ascontiguousarray(Wo[cols, :]),
            qres=np.ascontiguousarray(strips),
            ident=ident,
            onesv=np.ones((128, HPC), np.float32),
        ))
    return in_maps


def assemble(results, LEN):
    NQB = LEN // 512
    attn = np.empty((BS, NH, LEN, LEN), np.float32)
    out = np.empty((BS, LEN, DM), np.float32)
    for c in range(NCORES):
        b, rank = c // RPG, c % RPG
        attn[b, rank * HPC:(rank + 1) * HPC] = results[c]["attn_o"]
        oo = results[c]["out_o"]
        for qb in range(NQB):
            out[b, qb * 512 + rank * 128: qb * 512 + (rank + 1) * 128, :] = \
                oo[qb * 128:(qb + 1) * 128]
    return out, attn


def kernel(q, k, v, Wq, Wk, Wv, Wo):
    q = np.asarray(q, np.float32)
    k = np.asarray(k, np.float32)
    v = np.asarray(v, np.float32)
    LEN = q.shape[1]
    nc = _get_nc(LEN)
    in_maps = make_in_maps(q, k, v, np.asarray(Wq, np.float32),
                           np.asarray(Wk, np.float32), np.asarray(Wv, np.float32),
                           np.asarray(Wo, np.float32), LEN)
    res = run_bass_kernel_spmd(nc, in_maps, core_ids=list(range(NCORES)))
    out, attn = assemble(res.results, LEN)
    return (out, attn)


# revision 22
# speedup vs baseline: 82.1752x; 82.1752x over previous
"""Multi-head attention (bs=2, len=2048, d_model=1024, 16 heads) on 8 TRN2 cores.

Sharding: batch across 2 groups of 4 cores; 4 heads per core within a group.
Per core:
  - inputs fed host-transposed: qT/kT/vT [1024, 2048] (d_model on partitions)
  - projections qhT/khT/vhT [256, 2048] computed with psum-resident accumulation
  - vhT PE-transposed into V_ext [2048, 4*65] (65th col = ones -> colsum ride-along)
  - per (q-block 512, head): S^T = K_h Q_h^T in [k, q] layout, exp on ACT -> P^T,
    O_ext = [V|1]^T P^T accumulated over k (row 64 = softmax denominators)
  - P^T blocks PE-transposed back to [q, k]; PSUM->SBUF copy fused with the
    1/denominator per-partition scale -> attn output written in natural layout
  - O normalized via PE broadcast of the reciprocal row, fc partials per q-block
    ReduceScattered (add) within the 4-core group -> each core owns a 128-row strip
  - residual added post-RS; host reassembles strips.
All matmuls in float32r (TF32-like fast path) except the fp32 broadcast;
PE transposes are bit-exact permutations.
"""

import sys

sys.path.insert(0, "/opt/trn_rl_repo")

import numpy as np

import concourse.bacc as bacc
import concourse.tile as tile
from concourse import mybir
from concourse.bass_utils import run_bass_kernel_spmd

F32 = mybir.dt.float32
F32R = mybir.dt.float32r
AF = mybir.ActivationFunctionType

BS = 2
DM = 1024
NH = 16
DK = 64
NCORES = 8
GROUPS = [[0, 1, 2, 3], [4, 5, 6, 7]]
HPC = 4          # heads per core
RPG = 4          # ranks per group


def build_nc(LEN=2048, single=False):
    # single=True: replace the ReduceScatter with a local DMA so the module
    # has no collectives -> usable with TimelineSim (cost-model profiling).
    NKI = LEN // 128       # k chunks
    NQB = LEN // 512       # q blocks
    NDC = DM // 128        # d_model chunks
    WCOLS = HPC * DK       # 256: projection output columns for this core's heads

    nc = bacc.Bacc("TRN2", target_bir_lowering=False)

    qT = nc.declare_dram_parameter("qT", [DM, LEN], F32R, isOutput=False)
    kT = nc.declare_dram_parameter("kT", [DM, LEN], F32R, isOutput=False)
    vT = nc.declare_dram_parameter("vT", [DM, LEN], F32R, isOutput=False)
    wq = nc.declare_dram_parameter("wq", [DM, WCOLS], F32R, isOutput=False)
    wk = nc.declare_dram_parameter("wk", [DM, WCOLS], F32R, isOutput=False)
    wv = nc.declare_dram_parameter("wv", [DM, WCOLS], F32R, isOutput=False)
    wo = nc.declare_dram_parameter("wo", [WCOLS, DM], F32R, isOutput=False)
    qres = nc.declare_dram_parameter("qres", [NQB * 128, DM], F32, isOutput=False)
    ident = nc.declare_dram_parameter("ident", [128, 128], F32, isOutput=False)
    onesv = nc.declare_dram_parameter("onesv", [128, HPC], F32R, isOutput=False)

    attn_o = nc.declare_dram_parameter("attn_o", [HPC, LEN, LEN], F32, isOutput=True)
    out_o = nc.declare_dram_parameter("out_o", [NQB * 128, DM], F32, isOutput=True)

    with tile.TileContext(nc) as tc:
        with (
            tc.tile_pool(name="sb", bufs=1) as sb,
            tc.tile_pool(name="dram", bufs=1, space="DRAM") as dram,
        ):
            identF = sb.tile([128, 128], F32, tag="identF")
            nc.sync.dma_start(identF[:], ident[:])
            identR = sb.tile([128, 128], F32R, tag="identR")
            nc.sync.dma_start(identR[:], ident[:].bitcast(F32R))
            onesF = sb.tile([65, 128], F32, tag="onesF")
            nc.vector.memset(onesF[:], 1.0)

            # per-head Wo rows at partition base 0 for the fc
            wo4 = []
            for h in range(HPC):
                t = sb.tile([64, DM], F32R, tag="wo4", bufs=HPC, name=f"wo4_{h}")
                nc.sync.dma_start(t[:], wo[h * 64:(h + 1) * 64, :])
                wo4.append(t)

            # ---------------- PSUM pools (shared by all phases) ----------------
            with (
                tc.tile_pool(name="pss", bufs=2, space="PSUM") as pss,    # proj / S / fc
                tc.tile_pool(name="pso", bufs=2, space="PSUM") as pso,    # O_ext
                tc.tile_pool(name="pst", bufs=2, space="PSUM") as pst,    # transposes + bc + rc
            ):
                # ---- projections, column-chunked on the shared "s" psum tag.
                # Order v, k, q: attention q-block 0 can start as soon as the
                # first q column-chunk lands, overlapping the q-projection tail.
                qhT = [None, None]  # [128, LEN]: heads (0,1),(2,3), dims on partitions
                khT = [None, None]
                vhT = [None, None]  # F32, pre-transpose staging
                for name, src, w in (("v", vT, wv), ("k", kT, wk), ("q", qT, wq)):
                    f32r = name != "v"
                    dsts = []
                    for t in range(2):
                        dsts.append(sb.tile(
                            [128, LEN], F32R if f32r else F32,
                            tag="attn" if name == "v" else f"{name}hT{t}",
                            bufs=3 if name == "v" else 1,
                            name=f"{name}hT{t}"))
                    if name == "v":
                        vhT = dsts
                    elif name == "k":
                        khT = dsts
                    else:
                        qhT = dsts
                    wts = []
                    for d in range(NDC):
                        wt = sb.tile([128, WCOLS], F32R, tag="w", bufs=NDC,
                                     name=f"w_{name}{d}")
                        nc.sync.dma_start(wt[:], w[d * 128:(d + 1) * 128, :])
                        wts.append(wt)
                    for qc in range(LEN // 512):
                        s_ps = pss.tile([128, 1024], F32, tag="s",
                                        name=f"pp_{name}{qc}")
                        for d in range(NDC):
                            xt = sb.tile([128, 512], F32R, tag="xs", bufs=6,
                                         name=f"x_{name}{qc}_{d}")
                            nc.sync.dma_start(
                                xt[:], src[d * 128:(d + 1) * 128,
                                           qc * 512:(qc + 1) * 512])
                            for t in range(2):
                                nc.tensor.matmul(
                                    s_ps[:, t * 512:(t + 1) * 512],
                                    wts[d][:, t * 128:(t + 1) * 128],
                                    xt[:],
                                    start=(d == 0), stop=(d == NDC - 1),
                                )
                        for t in range(2):
                            nc.scalar.copy(
                                dsts[t][:, qc * 512:(qc + 1) * 512],
                                s_ps[:, t * 512:(t + 1) * 512])
                # V_ext: [128 seq, 4*65] per seq chunk; col h*65+64 = ones
                vext = []
                for sc in range(NKI):
                    t = sb.tile([128, HPC * 65], F32R, tag="vext", bufs=NKI,
                                name=f"vext{sc}")
                    nc.sync.dma_start(t[:, 64::65], onesv[:])
                    vext.append(t)
                for t in range(2):
                    for scg in range(NKI // 4):
                        tp = pst.tile([128, 512], F32, tag="t", name=f"vt{t}_{scg}")
                        for j in range(4):
                            sc = scg * 4 + j
                            nc.tensor.matmul(
                                tp[:, j * 128:(j + 1) * 128],
                                vhT[t][:, sc * 128:(sc + 1) * 128],
                                identF[:], is_transpose=True,
                                start=(j == 0), stop=(j == 3),
                            )
                        for j in range(4):
                            sc = scg * 4 + j
                            for hh in range(2):
                                h = 2 * t + hh
                                nc.scalar.copy(
                                    vext[sc][:, h * 65:h * 65 + 64],
                                    tp[:, j * 128 + hh * 64:j * 128 + hh * 64 + 64],
                                )

                # ---------------- Phase 2: attention ----------------
                for qb in range(NQB):
                    otn = []   # per head normalized O^T [64, 512]
                    for h in range(HPC):
                        hp, hoff = h // 2, (h % 2) * 64
                        o_ps = pso.tile([65, 512], F32, tag="o", name=f"o{qb}_{h}")
                        pts = []   # per ki: (tile, col offset)
                        for kg in range(NKI // 2):
                            s_ps = pss.tile([128, 1024], F32, tag="s",
                                            name=f"s{qb}_{h}_{kg}")
                            for j in range(2):
                                ki = 2 * kg + j
                                nc.tensor.matmul(
                                    s_ps[:, j * 512:(j + 1) * 512],
                                    khT[hp][hoff:hoff + 64, ki * 128:(ki + 1) * 128],
                                    qhT[hp][hoff:hoff + 64, qb * 512:(qb + 1) * 512],
                                    start=True, stop=True,
                                )
                            # one batched exp over both k-chunks (amortizes the
                            # PSUM access latency of the ACT instruction)
                            pt = sb.tile([128, 1024], F32R, tag="pt", bufs=14,
                                         name=f"pt{qb}_{h}_{kg}")
                            nc.scalar.activation(pt[:], s_ps[:], AF.Exp, scale=0.125)
                            for j in range(2):
                                ki = 2 * kg + j
                                nc.tensor.matmul(
                                    o_ps[:],
                                    vext[ki][:, h * 65:(h + 1) * 65],
                                    pt[:, j * 512:(j + 1) * 512],
                                    start=(ki == 0), stop=(ki == NKI - 1),
                                )
                                pts.append((pt, j * 512))

                        # reciprocals of colsums (row 64), still q-on-free
                        rrow = sb.tile([65, 512], F32, tag="rrow", bufs=1,
                                       name=f"rrow{qb}_{h}")
                        nc.vector.reciprocal(rrow[64:65, :], o_ps[64:65, :])
                        # broadcast recip across partitions 0..63 (fp32, exact)
                        bc = pst.tile([64, 512], F32, tag="t", name=f"bc{qb}_{h}")
                        nc.tensor.matmul(bc[:], onesF[64:65, 0:64],
                                         rrow[64:65, :], start=True, stop=True)
                        bcs = sb.tile([64, 512], F32, tag="bcs", bufs=1,
                                      name=f"bcs{qb}_{h}")
                        nc.scalar.copy(bcs[:], bc[:])
                        on = sb.tile([64, 512], F32R, tag="otn", bufs=HPC,
                                     name=f"otn{qb}_{h}")
                        nc.vector.tensor_mul(on[:], o_ps[0:64, :], bcs[:])
                        otn.append(on)
                        # transpose recip row -> [128, 4] q-on-partition
                        rc_ps = pst.tile([128, 4], F32, tag="t", name=f"rc{qb}_{h}")
                        for qs in range(4):
                            nc.tensor.matmul(
                                rc_ps[:, qs:qs + 1],
                                rrow[64:65, qs * 128:(qs + 1) * 128],
                                onesF[64:65, 0:1], is_transpose=True,
                                start=(qs == 0), stop=(qs == 3),
                            )
                        rcp = sb.tile([128, 4], F32, tag="rcp", bufs=2,
                                      name=f"rcp{qb}_{h}")
                        nc.scalar.copy(rcp[:], rc_ps[:])

                        # transpose P^T -> [q, k] blocks, scale-copy, DMA out
                        for qs in range(4):
                            asb = sb.tile([128, LEN], F32, tag="attn", bufs=3,
                                          name=f"attn{qb}_{h}_{qs}")
                            for kig in range(NKI // 4):
                                tp = pst.tile([128, 512], F32R, tag="t",
                                              name=f"t{qb}_{h}_{qs}_{kig}")
                                for j in range(4):
                                    ki = kig * 4 + j
                                    ptile, poff = pts[ki]
                                    nc.tensor.matmul(
                                        tp[:, j * 128:(j + 1) * 128],
                                        ptile[:, poff + qs * 128:poff + (qs + 1) * 128],
                                        identR[:], is_transpose=True,
                                        start=(j == 0), stop=(j == 3),
                                    )
                                if kig == 3:
                                    # offload 1/4 of the scale-copies to ACT
                                    nc.scalar.mul(
                                        asb[:, kig * 512:(kig + 1) * 512],
                                        tp[:].bitcast(F32),
                                        rcp[:, qs:qs + 1],
                                    )
                                else:
                                    nc.vector.tensor_scalar_mul(
                                        asb[:, kig * 512:(kig + 1) * 512],
                                        tp[:].bitcast(F32),
                                        rcp[:, qs:qs + 1],
                                    )
                            nc.sync.dma_start(
                                attn_o[h, qb * 512 + qs * 128:qb * 512 + (qs + 1) * 128, :],
                                asb[:],
                            )

                    # ---- fc partial for this q-block + ReduceScatter ----
                    partial = dram.tile([512, DM], F32, tag="partial", bufs=NQB,
                                        name=f"partial{qb}")
                    for qs in range(4):
                        psb = sb.tile([128, DM], F32, tag="psb", bufs=2,
                                      name=f"psb{qb}_{qs}")
                        fp = pss.tile([128, 1024], F32, tag="s",
                                      name=f"fc{qb}_{qs}")
                        for dm_i in range(2):
                            for h in range(HPC):
                                nc.tensor.matmul(
                                    fp[:, dm_i * 512:(dm_i + 1) * 512],
                                    otn[h][:, qs * 128:(qs + 1) * 128],
                                    wo4[h][:, dm_i * 512:(dm_i + 1) * 512],
                                    start=(h == 0), stop=(h == HPC - 1),
                                )
                        if qs % 2 == 0:
                            nc.vector.tensor_copy(psb[:], fp[:])
                        else:
                            nc.scalar.copy(psb[:], fp[:])
                        nc.sync.dma_start(
                            partial[qs * 128:(qs + 1) * 128, :], psb[:])
                    rsout = dram.tile([128, DM], F32, tag="rsout", bufs=NQB,
                                      name=f"rsout{qb}")
                    if single:
                        nc.sync.dma_start(rsout[:], partial[0:128, :])
                    else:
                        nc.gpsimd.collective_compute(
                            "ReduceScatter", mybir.AluOpType.add,
                            replica_groups=GROUPS,
                            ins=[partial.opt()], outs=[rsout.opt()],
                        )
                    rsb = sb.tile([128, DM], F32, tag="rsb", bufs=1,
                                  name=f"rsb{qb}")
                    nc.sync.dma_start(rsb[:], rsout[:])
                    osb = sb.tile([128, DM], F32, tag="osb", bufs=1,
                                  name=f"osb{qb}")
                    nc.sync.dma_start(osb[:], qres[qb * 128:(qb + 1) * 128, :])
                    nc.vector.tensor_add(osb[:], rsb[:], osb[:])
                    nc.sync.dma_start(out_o[qb * 128:(qb + 1) * 128, :], osb[:])

    nc.compile()
    return nc


_NC_CACHE = {}


def _get_nc(LEN):
    if LEN not in _NC_CACHE:
        _NC_CACHE[LEN] = build_nc(LEN)
    return _NC_CACHE[LEN]


def make_in_maps(q, k, v, Wq, Wk, Wv, Wo, LEN):
    NQB = LEN // 512
    ident = np.eye(128, dtype=np.float32)
    in_maps = []
    for c in range(NCORES):
        b, rank = c // RPG, c % RPG
        h0 = rank * HPC
        cols = slice(h0 * DK, (h0 + HPC) * DK)
        strips = np.concatenate(
            [q[b, qb * 512 + rank * 128: qb * 512 + (rank + 1) * 128, :]
             for qb in range(NQB)], axis=0)
        in_maps.append(dict(
            qT=np.ascontiguousarray(q[b].T),
            kT=np.ascontiguousarray(k[b].T),
            vT=np.ascontiguousarray(v[b].T),
            wq=np.ascontiguousarray(Wq[:, cols]),
            wk=np.ascontiguousarray(Wk[:, cols]),
            wv=np.ascontiguousarray(Wv[:, cols]),
            wo=np.ascontiguousarray(Wo[cols, :]),
            qres=np.ascontiguousarray(strips),
            ident=ident,
            onesv=np.ones((128, HPC), np.float32),
        ))
    return in_maps


def assemble(results, LEN):
    NQB = LEN // 512
    attn = np.empty((BS, NH, LEN, LEN), np.float32)
    out = np.empty((BS, LEN, DM), np.float32)
    for c in range(NCORES):
        b, rank = c // RPG, c % RPG
        attn[b, rank * HPC:(rank + 1) * HPC] = results[c]["attn_o"]
        oo = results[c]["out_o"]
        for qb in range(NQB):
            out[b, qb * 512 + rank * 128: qb * 512 + (rank + 1) * 128, :] = \
                oo[qb * 128:(qb + 1) * 128]
    return out, attn


def kernel(q, k, v, Wq, Wk, Wv, Wo):
    q = np.asarray(q, np.float32)
    k = np.asarray(k, np.float32)
    v = np.asarray(v, np.float32)
    LEN = q.shape[1]
    nc = _get_nc(LEN)
    in_maps = make_in_maps(q, k, v, np.asarray(Wq, np.float32),
                           np.asarray(Wk, np.float32), np.asarray(Wv, np.float32),
                           np.asarray(Wo, np.float32), LEN)
    res = run_bass_kernel_spmd(nc, in_maps, core_ids=list(range(NCORES)))
    out, attn = assemble(res.results, LEN)
    return (out, attn)


# revision 23
# speedup vs baseline: 84.9849x; 1.0342x over previous
"""Multi-head attention (bs=2, len=2048, d_model=1024, 16 heads) on 8 TRN2 cores.

Sharding: batch across 2 groups of 4 cores; 4 heads per core within a group.
Per core:
  - inputs fed host-transposed: qT/kT/vT [1024, 2048] (d_model on partitions)
  - projections qhT/khT/vhT [256, 2048] computed with psum-resident accumulation
  - vhT PE-transposed into V_ext [2048, 4*65] (65th col = ones -> colsum ride-along)
  - per (q-block 512, head): S^T = K_h Q_h^T in [k, q] layout, exp on ACT -> P^T,
    O_ext = [V|1]^T P^T accumulated over k (row 64 = softmax denominators)
  - P^T blocks PE-transposed back to [q, k]; PSUM->SBUF copy fused with the
    1/denominator per-partition scale -> attn output written in natural layout
  - O normalized via PE broadcast of the reciprocal row, fc partials per q-block
    ReduceScattered (add) within the 4-core group -> each core owns a 128-row strip
  - residual added post-RS; host reassembles strips.
All matmuls in float32r (TF32-like fast path) except the fp32 broadcast;
PE transposes are bit-exact permutations.
"""

import sys

sys.path.insert(0, "/opt/trn_rl_repo")

import numpy as np

import concourse.bacc as bacc
import concourse.tile as tile
from concourse import mybir
from concourse.bass_utils import run_bass_kernel_spmd

F32 = mybir.dt.float32
F32R = mybir.dt.float32r
AF = mybir.ActivationFunctionType

BS = 2
DM = 1024
NH = 16
DK = 64
NCORES = 8
GROUPS = [[0, 1, 2, 3], [4, 5, 6, 7]]
HPC = 4          # heads per core
RPG = 4          # ranks per group


def build_nc(LEN=2048, single=False):
    # single=True: replace the ReduceScatter with a local DMA so the module
    # has no collectives -> usable with TimelineSim (cost-model profiling).
    NKI = LEN // 128       # k chunks
    NQB = LEN // 512       # q blocks
    NDC = DM // 128        # d_model chunks
    WCOLS = HPC * DK       # 256: projection output columns for this core's heads

    nc = bacc.Bacc("TRN2", target_bir_lowering=False)

    qT = nc.declare_dram_parameter("qT", [DM, LEN], F32R, isOutput=False)
    kT = nc.declare_dram_parameter("kT", [DM, LEN], F32R, isOutput=False)
    vT = nc.declare_dram_parameter("vT", [DM, LEN], F32R, isOutput=False)
    wq = nc.declare_dram_parameter("wq", [DM, WCOLS], F32R, isOutput=False)
    wk = nc.declare_dram_parameter("wk", [DM, WCOLS], F32R, isOutput=False)
    wv = nc.declare_dram_parameter("wv", [DM, WCOLS], F32R, isOutput=False)
    wo = nc.declare_dram_parameter("wo", [WCOLS, DM], F32R, isOutput=False)
    qres = nc.declare_dram_parameter("qres", [NQB * 128, DM], F32, isOutput=False)
    ident = nc.declare_dram_parameter("ident", [128, 128], F32, isOutput=False)
    onesv = nc.declare_dram_parameter("onesv", [128, HPC], F32R, isOutput=False)

    attn_o = nc.declare_dram_parameter("attn_o", [HPC, LEN, LEN], F32, isOutput=True)
    out_o = nc.declare_dram_parameter("out_o", [NQB * 128, DM], F32, isOutput=True)

    with tile.TileContext(nc) as tc:
        with (
            tc.tile_pool(name="sb", bufs=1) as sb,
            tc.tile_pool(name="dram", bufs=1, space="DRAM") as dram,
        ):
            identF = sb.tile([128, 128], F32, tag="identF")
            nc.sync.dma_start(identF[:], ident[:])
            identR = sb.tile([128, 128], F32R, tag="identR")
            nc.sync.dma_start(identR[:], ident[:].bitcast(F32R))
            onesF = sb.tile([65, 128], F32, tag="onesF")
            nc.vector.memset(onesF[:], 1.0)

            # per-head Wo rows at partition base 0 for the fc
            wo4 = []
            for h in range(HPC):
                t = sb.tile([64, DM], F32R, tag="wo4", bufs=HPC, name=f"wo4_{h}")
                nc.sync.dma_start(t[:], wo[h * 64:(h + 1) * 64, :])
                wo4.append(t)

            # ---------------- PSUM pools (shared by all phases) ----------------
            with (
                tc.tile_pool(name="pss", bufs=2, space="PSUM") as pss,    # proj / S / fc
                tc.tile_pool(name="pso", bufs=2, space="PSUM") as pso,    # O_ext
                tc.tile_pool(name="pst", bufs=2, space="PSUM") as pst,    # transposes + bc + rc
            ):
                # ---- projections, column-chunked on the shared "s" psum tag.
                # Order v, k, q: attention q-block 0 can start as soon as the
                # first q column-chunk lands, overlapping the q-projection tail.
                qhT = [None, None]  # [128, LEN]: heads (0,1),(2,3), dims on partitions
                khT = [None, None]
                vhT = [None, None]  # F32, pre-transpose staging
                for name, src, w in (("v", vT, wv), ("k", kT, wk), ("q", qT, wq)):
                    f32r = name != "v"
                    dsts = []
                    for t in range(2):
                        dsts.append(sb.tile(
                            [128, LEN], F32R if f32r else F32,
                            tag="attn" if name == "v" else f"{name}hT{t}",
                            bufs=3 if name == "v" else 1,
                            name=f"{name}hT{t}"))
                    if name == "v":
                        vhT = dsts
                    elif name == "k":
                        khT = dsts
                    else:
                        qhT = dsts
                    wts = []
                    for d in range(NDC):
                        wt = sb.tile([128, WCOLS], F32R, tag="w", bufs=NDC,
                                     name=f"w_{name}{d}")
                        nc.sync.dma_start(wt[:], w[d * 128:(d + 1) * 128, :])
                        wts.append(wt)
                    for qc in range(LEN // 512):
                        s_ps = pss.tile([128, 1024], F32, tag="s",
                                        name=f"pp_{name}{qc}")
                        for d in range(NDC):
                            xt = sb.tile([128, 512], F32R, tag="xs", bufs=6,
                                         name=f"x_{name}{qc}_{d}")
                            nc.sync.dma_start(
                                xt[:], src[d * 128:(d + 1) * 128,
                                           qc * 512:(qc + 1) * 512])
                            for t in range(2):
                                nc.tensor.matmul(
                                    s_ps[:, t * 512:(t + 1) * 512],
                                    wts[d][:, t * 128:(t + 1) * 128],
                                    xt[:],
                                    start=(d == 0), stop=(d == NDC - 1),
                                )
                        for t in range(2):
                            nc.scalar.copy(
                                dsts[t][:, qc * 512:(qc + 1) * 512],
                                s_ps[:, t * 512:(t + 1) * 512])
                # V_ext: [128 seq, 4*65] per seq chunk; col h*65+64 = ones
                vext = []
                for sc in range(NKI):
                    t = sb.tile([128, HPC * 65], F32R, tag="vext", bufs=NKI,
                                name=f"vext{sc}")
                    nc.sync.dma_start(t[:, 64::65], onesv[:])
                    vext.append(t)
                for t in range(2):
                    for scg in range(NKI // 4):
                        tp = pst.tile([128, 512], F32, tag="t", name=f"vt{t}_{scg}")
                        for j in range(4):
                            sc = scg * 4 + j
                            nc.tensor.matmul(
                                tp[:, j * 128:(j + 1) * 128],
                                vhT[t][:, sc * 128:(sc + 1) * 128],
                                identF[:], is_transpose=True,
                                start=(j == 0), stop=(j == 3),
                            )
                        for j in range(4):
                            sc = scg * 4 + j
                            for hh in range(2):
                                h = 2 * t + hh
                                nc.scalar.copy(
                                    vext[sc][:, h * 65:h * 65 + 64],
                                    tp[:, j * 128 + hh * 64:j * 128 + hh * 64 + 64],
                                )

                # ---------------- Phase 2: attention ----------------
                for qb in range(NQB):
                    otn = []   # per head normalized O^T [64, 512]
                    for h in range(HPC):
                        hp, hoff = h // 2, (h % 2) * 64
                        o_ps = pso.tile([65, 512], F32, tag="o", name=f"o{qb}_{h}")
                        pts = []   # per ki: (tile, col offset)
                        for kg in range(NKI // 2):
                            s_ps = pss.tile([128, 1024], F32, tag="s",
                                            name=f"s{qb}_{h}_{kg}")
                            for j in range(2):
                                ki = 2 * kg + j
                                nc.tensor.matmul(
                                    s_ps[:, j * 512:(j + 1) * 512],
                                    khT[hp][hoff:hoff + 64, ki * 128:(ki + 1) * 128],
                                    qhT[hp][hoff:hoff + 64, qb * 512:(qb + 1) * 512],
                                    start=True, stop=True,
                                )
                            # one batched exp over both k-chunks (amortizes the
                            # PSUM access latency of the ACT instruction)
                            pt = sb.tile([128, 1024], F32R, tag="pt", bufs=16,
                                         name=f"pt{qb}_{h}_{kg}")
                            nc.scalar.activation(pt[:], s_ps[:], AF.Exp, scale=0.125)
                            for j in range(2):
                                ki = 2 * kg + j
                                nc.tensor.matmul(
                                    o_ps[:],
                                    vext[ki][:, h * 65:(h + 1) * 65],
                                    pt[:, j * 512:(j + 1) * 512],
                                    start=(ki == 0), stop=(ki == NKI - 1),
                                )
                                pts.append((pt, j * 512))

                        # reciprocals of colsums (row 64), still q-on-free
                        rrow = sb.tile([65, 512], F32, tag="rrow", bufs=1,
                                       name=f"rrow{qb}_{h}")
                        nc.vector.reciprocal(rrow[64:65, :], o_ps[64:65, :])
                        # broadcast recip across partitions 0..63 (fp32, exact)
                        bc = pst.tile([64, 512], F32, tag="t", name=f"bc{qb}_{h}")
                        nc.tensor.matmul(bc[:], onesF[64:65, 0:64],
                                         rrow[64:65, :], start=True, stop=True)
                        bcs = sb.tile([64, 512], F32, tag="bcs", bufs=1,
                                      name=f"bcs{qb}_{h}")
                        nc.scalar.copy(bcs[:], bc[:])
                        on = sb.tile([64, 512], F32R, tag="otn", bufs=HPC,
                                     name=f"otn{qb}_{h}")
                        nc.vector.tensor_mul(on[:], o_ps[0:64, :], bcs[:])
                        otn.append(on)
                        # transpose recip row -> [128, 4] q-on-partition
                        rc_ps = pst.tile([128, 4], F32, tag="t", name=f"rc{qb}_{h}")
                        for qs in range(4):
                            nc.tensor.matmul(
                                rc_ps[:, qs:qs + 1],
                                rrow[64:65, qs * 128:(qs + 1) * 128],
                                onesF[64:65, 0:1], is_transpose=True,
                                start=(qs == 0), stop=(qs == 3),
                            )
                        rcp = sb.tile([128, 4], F32, tag="rcp", bufs=2,
                                      name=f"rcp{qb}_{h}")
                        nc.scalar.copy(rcp[:], rc_ps[:])

                        # transpose P^T -> [q, k] blocks, scale-copy, DMA out
                        for qs in range(4):
                            asb = sb.tile([128, LEN], F32, tag="attn", bufs=3,
                                          name=f"attn{qb}_{h}_{qs}")
                            for kig in range(NKI // 4):
                                tp = pst.tile([128, 512], F32R, tag="t",
                                              name=f"t{qb}_{h}_{qs}_{kig}")
                                for j in range(4):
                                    ki = kig * 4 + j
                                    ptile, poff = pts[ki]
                                    nc.tensor.matmul(
                                        tp[:, j * 128:(j + 1) * 128],
                                        ptile[:, poff + qs * 128:poff + (qs + 1) * 128],
                                        identR[:], is_transpose=True,
                                        start=(j == 0), stop=(j == 3),
                                    )
                                if kig == 3:
                                    # offload 1/4 of the scale-copies to ACT
                                    nc.scalar.mul(
                                        asb[:, kig * 512:(kig + 1) * 512],
                                        tp[:].bitcast(F32),
                                        rcp[:, qs:qs + 1],
                                    )
                                else:
                                    nc.vector.tensor_scalar_mul(
                                        asb[:, kig * 512:(kig + 1) * 512],
                                        tp[:].bitcast(F32),
                                        rcp[:, qs:qs + 1],
                                    )
                            nc.sync.dma_start(
                                attn_o[h, qb * 512 + qs * 128:qb * 512 + (qs + 1) * 128, :],
                                asb[:],
                            )

                    # ---- fc partial for this q-block + ReduceScatter ----
                    partial = dram.tile([512, DM], F32, tag="partial", bufs=NQB,
                                        name=f"partial{qb}")
                    for qs in range(4):
                        psb = sb.tile([128, DM], F32, tag="psb", bufs=2,
                                      name=f"psb{qb}_{qs}")
                        fp = pss.tile([128, 1024], F32, tag="s",
                                      name=f"fc{qb}_{qs}")
                        for dm_i in range(2):
                            for h in range(HPC):
                                nc.tensor.matmul(
                                    fp[:, dm_i * 512:(dm_i + 1) * 512],
                                    otn[h][:, qs * 128:(qs + 1) * 128],
                                    wo4[h][:, dm_i * 512:(dm_i + 1) * 512],
                                    start=(h == 0), stop=(h == HPC - 1),
                                )
                        if qs % 2 == 0:
                            nc.vector.tensor_copy(psb[:], fp[:])
                        else:
                            nc.scalar.copy(psb[:], fp[:])
                        nc.sync.dma_start(
                            partial[qs * 128:(qs + 1) * 128, :], psb[:])
                    rsout = dram.tile([128, DM], F32, tag="rsout", bufs=NQB,
                                      name=f"rsout{qb}")
                    if single:
                        nc.sync.dma_start(rsout[:], partial[0:128, :])
                    else:
                        nc.gpsimd.collective_compute(
                            "ReduceScatter", mybir.AluOpType.add,
                            replica_groups=GROUPS,
                            ins=[partial.opt()], outs=[rsout.opt()],
                        )
                    rsb = sb.tile([128, DM], F32, tag="rsb", bufs=1,
                                  name=f"rsb{qb}")
                    nc.sync.dma_start(rsb[:], rsout[:])
                    osb = sb.tile([128, DM], F32, tag="osb", bufs=1,
                                  name=f"osb{qb}")
                    nc.sync.dma_start(osb[:], qres[qb * 128:(qb + 1) * 128, :])
                    nc.vector.tensor_add(osb[:], rsb[:], osb[:])
                    nc.sync.dma_start(out_o[qb * 128:(qb + 1) * 128, :], osb[:])

    nc.compile()
    return nc


_NC_CACHE = {}


def _get_nc(LEN):
    if LEN not in _NC_CACHE:
        _NC_CACHE[LEN] = build_nc(LEN)
    return _NC_CACHE[LEN]


def make_in_maps(q, k, v, Wq, Wk, Wv, Wo, LEN):
    NQB = LEN // 512
    ident = np.eye(128, dtype=np.float32)
    in_maps = []
    for c in range(NCORES):
        b, rank = c // RPG, c % RPG
        h0 = rank * HPC
        cols = slice(h0 * DK, (h0 + HPC) * DK)
        strips = np.concatenate(
            [q[b, qb * 512 + rank * 128: qb * 512 + (rank + 1) * 128, :]
             for qb in range(NQB)], axis=0)
        in_maps.append(dict(
            qT=np.ascontiguousarray(q[b].T),
            kT=np.ascontiguousarray(k[b].T),
            vT=np.ascontiguousarray(v[b].T),
            wq=np.ascontiguousarray(Wq[:, cols]),
            wk=np.ascontiguousarray(Wk[:, cols]),
            wv=np.ascontiguousarray(Wv[:, cols]),
            wo=np.ascontiguousarray(Wo[cols, :]),
            qres=np.ascontiguousarray(strips),
            ident=ident,
            onesv=np.ones((128, HPC), np.float32),
        ))
    return in_maps


def assemble(results, LEN):
    NQB = LEN // 512
    attn = np.empty((BS, NH, LEN, LEN), np.float32)
    out = np.empty((BS, LEN, DM), np.float32)
    for c in range(NCORES):
        b, rank = c // RPG, c % RPG
        attn[b, rank * HPC:(rank + 1) * HPC] = results[c]["attn_o"]
        oo = results[c]["out_o"]
        for qb in range(NQB):
            out[b, qb * 512 + rank * 128: qb * 512 + (rank + 1) * 128, :] = \
                oo[qb * 128:(qb + 1) * 128]
    return out, attn


def kernel(q, k, v, Wq, Wk, Wv, Wo):
    q = np.asarray(q, np.float32)
    k = np.asarray(k, np.float32)
    v = np.asarray(v, np.float32)
    LEN = q.shape[1]
    nc = _get_nc(LEN)
    in_maps = make_in_maps(q, k, v, np.asarray(Wq, np.float32),
                           np.asarray(Wk, np.float32), np.asarray(Wv, np.float32),
                           np.asarray(Wo, np.float32), LEN)
    res = run_bass_kernel_spmd(nc, in_maps, core_ids=list(range(NCORES)))
    out, attn = assemble(res.results, LEN)
    return (out, attn)
